# revision 1
# baseline (speedup 1.0000x reference)
"""Trainium2 Bass kernel for nn_MultiHeadAttention_41884521070801.

Sharding: tensor-parallel over heads (4 heads/core) x data-parallel over
batch (B=2) => 8 cores. Each core computes, for its batch element and its
4 heads: QKV projections (+RoPE), causal softmax attention (flash-style,
transposed-scores layout so no transposes are needed on-device), and its
partial output projection (rows of Wo^T). Host sums the 4 partial outputs
per batch element.

All matmuls run in bf16 with fp32 PSUM accumulation. RoPE and softmax
statistics are computed in fp32.
"""

import math

import numpy as np
import ml_dtypes

import concourse.bacc as bacc
import concourse.tile as tile
from concourse import mybir
from concourse.bass_utils import run_bass_kernel_spmd

N_CORES = 8
B = 2
S = 2048
D = 2048
H = 16
HD = 128          # head dim
HLOC = 4          # heads per core
DLOC = HLOC * HD  # 512, per-core slice of the concat-head dim
QCH = 512         # q chunk size
NQC = S // QCH    # 4
NKB = S // 128    # 16 k-blocks
NEB = D // 128    # 16 e-blocks (contraction blocks for projections)
ROPE_THETA = 10000.0
NEG = -1.0e30

F32 = mybir.dt.float32
BF16 = mybir.dt.bfloat16

_BUILD_CACHE = {}

# ablation flags (timing experiments only; correctness requires all True)
FLAGS = {
    "io_dma": True,    # xT chunk loads + output stores
    "exp": True,       # ACT exp (else DVE copy)
    "sums": True,      # row-sum matmuls + normalization
    "mask": True,      # causal ctri adds
    "rope": True,      # rope DVE/ACT work (else direct copy)
    "timing_io": False,  # all data in internal DRAM, tiny external I/O
    "out_gpsimd": False,  # issue output stores on the gpsimd queue
    "act_copies": True,  # psum evacuation copies on ACT (else DVE)
    "attn": True,      # attention phase
    "wo": True,        # output projection phase
    "proj": True,      # QKV projection phase
}


def _emit_consts(nc, tc, pools, tensors):
    """Emit the one-time constant/weight loads."""
    (consts, resid, xc_pool, ps_pool, work, p_pool, rb_pool, oc_pool,
     qcur_pool, ocur_pool) = pools
    (xT, wqT, wkT, wvT, woT, cosT, sinT, rT, amB, ctri, ident, outp) = tensors
    if True:
        consts.wq = consts.tile([128, NEB, DLOC], BF16, tag="wq", name="wq")
        consts.wk = consts.tile([128, NEB, DLOC], BF16, tag="wk", name="wk")
        consts.wv = consts.tile([128, NEB, DLOC], BF16, tag="wv", name="wv")
        consts.wo = consts.tile([128, HLOC, D], BF16, tag="wo", name="wo")
        for e in range(NEB):
            nc.sync.dma_start(out=consts.wq[:, e, :], in_=wqT[e])
            nc.sync.dma_start(out=consts.wk[:, e, :], in_=wkT[e])
            nc.sync.dma_start(out=consts.wv[:, e, :], in_=wvT[e])
        for hh in range(HLOC):
            nc.sync.dma_start(out=consts.wo[:, hh, :], in_=woT[hh])
        consts.cos = consts.tile([128, S], F32, tag="cos", name="cos")
        consts.sin = consts.tile([128, S], F32, tag="sin", name="sin")
        nc.sync.dma_start(out=consts.cos, in_=cosT[:])
        nc.sync.dma_start(out=consts.sin, in_=sinT[:])
        consts.rT = consts.tile([128, HD], BF16, tag="rT", name="rTs")
        nc.sync.dma_start(out=consts.rT, in_=rT[:])
        consts.amB = consts.tile([128, NKB], F32, tag="amB", name="amBs")
        nc.sync.dma_start(out=consts.amB, in_=amB[:])
        consts.ctri = consts.tile([128, 4, QCH], BF16, tag="ctri", name="ctri")
        nc.sync.dma_start(out=consts.ctri, in_=ctri[:].rearrange("p (j q) -> p j q", j=4))
        consts.ident = consts.tile([128, 128], BF16, tag="ident", name="ident")
        nc.sync.dma_start(out=consts.ident, in_=ident[:])
        consts.ones_bf = consts.tile([128, 1], BF16, tag="ones_bf", name="ones_bf")
        nc.vector.memset(consts.ones_bf, 1.0)
        consts.ones_row = consts.tile([1, 128], F32, tag="ones_row", name="ones_row")
        nc.vector.memset(consts.ones_row, 1.0)
        # persistent activations (K and V must stay for the whole pass)
        consts.kro = [resid.tile([128, S], BF16, tag=f"kro{h}", name=f"kro{h}")
                      for h in range(HLOC)]
        consts.v = [resid.tile([128, DLOC], BF16, tag=f"v{kb}", name=f"v{kb}")
                    for kb in range(NKB)]


def _emit_body(nc, tc, pools, tensors):
    """Emit one full forward pass (consts already emitted).

    PE executes its instruction stream in order, so cross-engine consumers
    (rope, exp) are software-pipelined: matmuls that depend on another
    engine's output are emitted 1-2 producer-iterations late so the PE
    always has independent work queued.
    """
    (consts, resid, xc_pool, ps_pool, work, p_pool, rb_pool, oc_pool,
     qcur_pool, ocur_pool) = pools
    (xT, wqT, wkT, wvT, woT, cosT, sinT, rT, amB, ctri, ident, outp) = tensors

    wq, wk, wv, wo = consts.wq, consts.wk, consts.wv, consts.wo
    cos_s, sin_s, rT_s, amB_s, ctri_s = (
        consts.cos, consts.sin, consts.rT, consts.amB, consts.ctri)
    kro, v_s = consts.kro, consts.v

    def rope_pre(src_ps, qc):
        """ACT-copy psum -> bf16 sbuf (stage 1 of rope)."""
        qf = work.tile([128, QCH], BF16, tag="ropef", name="ropef", bufs=4)
        if FLAGS["act_copies"]:
            nc.scalar.copy(qf, src_ps)
        else:
            nc.vector.tensor_copy(qf, src_ps)
        return qf

    def rope_rot(qf):
        """PE rotate-half matmul (stage 2)."""
        rot = ps_pool.tile([128, QCH], F32, tag="ps", name="ps")
        nc.tensor.matmul(rot, lhsT=rT_s, rhs=qf, start=True, stop=True)
        return rot

    def rope_fin(qf, rot, dst_ap, qc):
        """DVE combine (stage 3)."""
        t1 = work.tile([128, QCH], F32, tag="ropet1", name="ropet1", bufs=3)
        nc.vector.tensor_mul(t1, qf, cos_s[:, qc * QCH:(qc + 1) * QCH])
        t2 = work.tile([128, QCH], F32, tag="ropet2", name="ropet2", bufs=3)
        nc.vector.tensor_mul(t2, rot, sin_s[:, qc * QCH:(qc + 1) * QCH])
        nc.vector.tensor_add(dst_ap, t1, t2)

    for qc in range(NQC):
        # ---- load x^T chunk: 16 tiles [128 e, 512 q] ----
        xc = []
        for e in range(NEB):
            t = xc_pool.tile([128, QCH], BF16, tag="xc", name="xc")
            if FLAGS["io_dma"]:
                nc.sync.dma_start(out=t, in_=xT[qc, e])
            else:
                nc.vector.memset(t, 0.001)
            xc.append(t)

        # ---- QKV projections, rope software-pipelined behind them ----
        qcur = []
        if FLAGS["proj"]:
            # chains: (weight, head, dst_ap) for Q then K
            chains = []
            for h in range(HLOC):
                qt = qcur_pool.tile([128, QCH], BF16, tag="qcur", name="qcur")
                qcur.append(qt)
                chains.append((wq, h, qt[:]))
            for h in range(HLOC):
                chains.append((wk, h, kro[h][:, qc * QCH:(qc + 1) * QCH]))

            pending = []  # (qf, pp, dst_ap) awaiting rot+fin
            def drain_pending():
                qf, dst_ap = pending.pop(0)
                rot = rope_rot(qf)
                rope_fin(qf, rot, dst_ap, qc)

            for (w_s, h, dst_ap) in chains:
                pp = ps_pool.tile([128, QCH], F32, tag="ps", name="ps")
                for e in range(NEB):
                    nc.tensor.matmul(
                        pp, lhsT=w_s[:, e, h * HD:(h + 1) * HD], rhs=xc[e],
                        start=(e == 0), stop=(e == NEB - 1))
                qf = rope_pre(pp, qc)
                pending.append((qf, dst_ap))
                if len(pending) >= 2:
                    drain_pending()

            # ---- V (natural [k, d] layout) ----
            for kb4 in range(4):
                kb = qc * 4 + kb4
                pp = ps_pool.tile([128, DLOC], F32, tag="ps", name="ps")
                for e in range(NEB):
                    nc.tensor.matmul(
                        pp, lhsT=xc[e][:, kb4 * 128:(kb4 + 1) * 128],
                        rhs=wv[:, e, :],
                        start=(e == 0), stop=(e == NEB - 1))
                if FLAGS["act_copies"]:
                    nc.scalar.copy(v_s[kb], pp)
                else:
                    nc.vector.tensor_copy(v_s[kb], pp)
                while pending:
                    drain_pending()
            while pending:
                drain_pending()
        else:
            for h in range(HLOC):
                qt = qcur_pool.tile([128, QCH], BF16, tag="qcur", name="qcur")
                qcur.append(qt)
                nc.vector.memset(qt, 0.01)

        # ---- attention: scores+exp pipelined 2 ahead of PV/sums ----
        nkb = 4 * qc + 4
        ocur = []
        norm_q = []

        def emit_normalize():
            h0, ops0, sps0, ot0 = norm_q.pop(0)
            if FLAGS["sums"]:
                # normalize: o = ops * (1/sums), broadcast along partitions
                # via a K=1 outer-product matmul
                r_row = rb_pool.tile([1, QCH], F32, tag="rrow", name="rrow")
                nc.vector.reciprocal(r_row, sps0)
                rb_ps = ps_pool.tile([128, QCH], F32, tag="ps", name="ps")
                nc.tensor.matmul(rb_ps, lhsT=consts.ones_row, rhs=r_row,
                                 start=True, stop=True)
                rb_sb = rb_pool.tile([128, QCH], F32, tag="rb", name="rb")
                nc.vector.tensor_copy(rb_sb, rb_ps)
                nc.vector.tensor_mul(ot0[:], ops0, rb_sb)
            else:
                nc.vector.tensor_copy(ot0[:], ops0)

        for h in range(HLOC if FLAGS["attn"] else 0):
            ops = ps_pool.tile([128, QCH], F32, tag="ps", name="ps")
            sps = ps_pool.tile([1, QCH], F32, tag="ps", name="ps")

            def emit_scores(kb):
                off = max(0, (kb - 4 * qc) * 128)
                diag = kb >= 4 * qc
                s_ps = ps_pool.tile([128, QCH], F32, tag="ps", name="ps")
                nc.tensor.matmul(
                    s_ps[:, off:], lhsT=kro[h][:, kb * 128:(kb + 1) * 128],
                    rhs=qcur[h][:, off:], start=True,
                    stop=not (FLAGS["mask"] and diag))
                if FLAGS["mask"] and diag:
                    # accumulate the additive causal triangle: I.T @ tri
                    j = kb - 4 * qc
                    nc.tensor.matmul(
                        s_ps[:, off:], lhsT=consts.ident, rhs=ctri_s[:, j, off:],
                        start=False, stop=True)
                p_sb = p_pool.tile([128, QCH], BF16, tag="p", name="p")
                if FLAGS["exp"]:
                    nc.scalar.activation(
                        p_sb[:, off:], s_ps[:, off:],
                        mybir.ActivationFunctionType.Exp,
                        bias=amB_s[:, kb:kb + 1], scale=1.0)
                else:
                    nc.vector.tensor_copy(p_sb[:, off:], s_ps[:, off:])
                return (p_sb, off)

            LOOKAHEAD = 2
            fifo = [emit_scores(kb) for kb in range(min(LOOKAHEAD, nkb))]
            for kb in range(nkb):
                if kb + LOOKAHEAD < nkb:
                    fifo.append(emit_scores(kb + LOOKAHEAD))
                p_sb, off = fifo.pop(0)
                nc.tensor.matmul(
                    ops[:, off:], lhsT=v_s[kb][:, h * HD:(h + 1) * HD],
                    rhs=p_sb[:, off:],
                    start=(kb == 0), stop=(kb == nkb - 1), skip_group_check=True)
                if FLAGS["sums"]:
                    nc.tensor.matmul(
                        sps[:, off:], lhsT=consts.ones_bf, rhs=p_sb[:, off:],
                        start=(kb == 0), stop=(kb == nkb - 1),
                        skip_group_check=True)
                if kb == 1 and norm_q:
                    # drain the previous head's normalize: its inputs are
                    # ready, so the PE bcast matmul does not stall, and the
                    # held psum banks free up early
                    emit_normalize()

            ot = ocur_pool.tile([128, QCH], BF16, tag="ocur", name="ocur")
            ocur.append(ot)
            norm_q.append((h, ops, sps, ot))
        while norm_q:
            emit_normalize()

        # ---- output projection for this chunk ----
        if not (FLAGS["wo"] and FLAGS["attn"]):
            continue
        for qb4 in range(QCH // 128):
            qb = qc * 4 + qb4
            for ec in range(D // QCH):
                op_ps = ps_pool.tile([128, QCH], F32, tag="ps", name="ps")
                for h in range(HLOC):
                    nc.tensor.matmul(
                        op_ps,
                        lhsT=ocur[h][:, qb4 * 128:(qb4 + 1) * 128],
                        rhs=wo[:, h, ec * QCH:(ec + 1) * QCH],
                        start=(h == 0), stop=(h == HLOC - 1))
                oc = oc_pool.tile([128, QCH], F32, tag="oc", name="oc")
                nc.vector.tensor_copy(oc, op_ps)
                if FLAGS["io_dma"]:
                    eng = nc.gpsimd if FLAGS["out_gpsimd"] else nc.sync
                    eng.dma_start(out=outp[qb, ec], in_=oc)


def build_nc(repeat=1):
    key = (repeat, tuple(sorted(FLAGS.items())))
    if key in _BUILD_CACHE:
        return _BUILD_CACHE[key]
    nc = bacc.Bacc("TRN2", target_bir_lowering=False, debug=False,
                   num_devices=N_CORES)
    if FLAGS["timing_io"]:
        kind = "Internal"
        dummy_in = nc.dram_tensor("dummy_in", [1, 4], F32, kind="ExternalInput")
        dummy_out = nc.dram_tensor("dummy_out", [1, 4], F32, kind="ExternalOutput")
    else:
        kind = "ExternalInput"
    xT = nc.dram_tensor("xT", [NQC, NEB, 128, QCH], BF16, kind=kind)
    wqT = nc.dram_tensor("wqT", [NEB, 128, DLOC], BF16, kind=kind)
    wkT = nc.dram_tensor("wkT", [NEB, 128, DLOC], BF16, kind=kind)
    wvT = nc.dram_tensor("wvT", [NEB, 128, DLOC], BF16, kind=kind)
    woT = nc.dram_tensor("woT", [HLOC, 128, D], BF16, kind=kind)
    cosT = nc.dram_tensor("cosT", [HD, S], F32, kind=kind)
    sinT = nc.dram_tensor("sinT", [HD, S], F32, kind=kind)
    rT = nc.dram_tensor("rT", [HD, HD], BF16, kind=kind)
    amB = nc.dram_tensor("amB", [128, NKB], F32, kind=kind)
    ctri = nc.dram_tensor("tri", [128, 4 * QCH], BF16, kind=kind)
    ident = nc.dram_tensor("ident", [128, 128], BF16, kind=kind)
    if FLAGS["timing_io"]:
        outp = nc.dram_tensor("outp", [S // 128, D // QCH, 128, QCH], F32,
                              kind="Internal")
    else:
        outp = nc.dram_tensor("outp", [S // 128, D // QCH, 128, QCH], F32,
                              kind="ExternalOutput")
    tensors = (xT, wqT, wkT, wvT, woT, cosT, sinT, rT, amB, ctri, ident, outp)

    from contextlib import ExitStack
    with tile.TileContext(nc) as tc, ExitStack() as ctx:
        consts = ctx.enter_context(tc.tile_pool(name="consts", bufs=1))
        resid = ctx.enter_context(tc.tile_pool(name="resid", bufs=1))
        xc_pool = ctx.enter_context(tc.tile_pool(name="xc", bufs=20))
        ps_pool = ctx.enter_context(tc.tile_pool(name="ps", bufs=8, space="PSUM"))
        work = ctx.enter_context(tc.tile_pool(name="work", bufs=2))
        p_pool = ctx.enter_context(tc.tile_pool(name="p", bufs=6))
        rb_pool = ctx.enter_context(tc.tile_pool(name="rb", bufs=2))
        oc_pool = ctx.enter_context(tc.tile_pool(name="oc", bufs=3))
        qcur_pool = ctx.enter_context(tc.tile_pool(name="qcur", bufs=8))
        ocur_pool = ctx.enter_context(tc.tile_pool(name="ocur", bufs=8))
        pools = (consts, resid, xc_pool, ps_pool, work, p_pool, rb_pool,
                 oc_pool, qcur_pool, ocur_pool)
        _emit_consts(nc, tc, pools, tensors)
        if FLAGS["timing_io"]:
            dsb = pools[4].tile([1, 4], F32, tag="dummy", name="dummy")
            nc.sync.dma_start(out=dsb, in_=dummy_in[:])
            nc.sync.dma_start(out=dummy_out[:], in_=dsb)
        if repeat == 1:
            _emit_body(nc, tc, pools, tensors)
        else:
            with tc.For_i(0, repeat, 1, hint_engines=(mybir.EngineType.PE, mybir.EngineType.DVE, mybir.EngineType.Activation)):
                _emit_body(nc, tc, pools, tensors)
    nc.compile()
    _BUILD_CACHE[key] = nc
    return nc


def make_core_inputs(hidden_states, attention_mask, Wq, Wk, Wv, Wo):
    """Host-side prep: returns list of 8 in_maps."""
    f32 = np.float32
    bf16 = ml_dtypes.bfloat16
    hidden_states = np.asarray(hidden_states, dtype=f32)
    attention_mask = np.asarray(attention_mask, dtype=f32)
    Wq = np.asarray(Wq, dtype=f32)
    Wk = np.asarray(Wk, dtype=f32)
    Wv = np.asarray(Wv, dtype=f32)
    Wo = np.asarray(Wo, dtype=f32)

    # rope tables, [hd, S] layout
    invf = 1.0 / (ROPE_THETA ** (np.arange(0, HD, 2, dtype=f32) / HD))
    t = np.arange(S, dtype=f32)
    fr = t[:, None] * invf[None, :]            # [S, hd/2]
    emb = np.concatenate([fr, fr], axis=-1)    # [S, hd]
    cosT = np.cos(emb).T.astype(f32).copy()    # [hd, S]
    sinT = np.sin(emb).T.astype(f32).copy()

    # rotate-half matrix: (R @ x)[i] = -x[i+64] (i<64), x[i-64] (i>=64)
    R = np.zeros((HD, HD), dtype=f32)
    half = HD // 2
    for i in range(half):
        R[i, i + half] = -1.0
        R[i + half, i] = 1.0
    rT = R.T.copy()

    # causal additive triangle for the diagonal 128x128 sub-block
    p = np.arange(128)[:, None]
    c = np.arange(QCH)[None, :]
    tri = np.zeros((128, 4, QCH), dtype=np.float32)
    for j in range(4):
        qrel = c - 128 * j
        tri[:, j, :] = np.where((qrel >= 0) & (qrel < 128) & (p > qrel), NEG, 0.0)
    tri = tri.reshape(128, 4 * QCH).astype(bf16)
    ident = np.eye(128, dtype=np.float32).astype(bf16)

    scale = 1.0 / math.sqrt(HD)
    in_maps = []
    for core in range(N_CORES):
        b = core // (N_CORES // B)
        hg = core % (N_CORES // B)
        rows = slice(hg * DLOC, (hg + 1) * DLOC)
        amv = np.where(attention_mask[b] == 0, NEG, attention_mask[b]).astype(f32)
        in_maps.append({
            "xT": np.ascontiguousarray(
                hidden_states[b].T.reshape(NEB, 128, NQC, QCH)
                .transpose(2, 0, 1, 3)).astype(bf16),
            "wqT": (Wq[rows, :] * scale).T.reshape(NEB, 128, DLOC).astype(bf16),
            "wkT": Wk[rows, :].T.reshape(NEB, 128, DLOC).astype(bf16),
            "wvT": Wv[rows, :].T.reshape(NEB, 128, DLOC).astype(bf16),
            "woT": Wo[:, rows].T.reshape(HLOC, 128, D).astype(bf16),
            "cosT": cosT,
            "sinT": sinT,
            "rT": rT.astype(bf16),
            "amB": amv.reshape(NKB, 128).T.copy(),
            "tri": tri,
            "ident": ident,
        })
    return in_maps


def kernel(**inputs):
    nc = build_nc()
    in_maps = make_core_inputs(**inputs)
    res = run_bass_kernel_spmd(nc, in_maps, list(range(N_CORES)))
    out = np.zeros((B, S, D), dtype=np.float32)
    ncb = N_CORES // B
    for core in range(N_CORES):
        r = res.results[core]["outp"]          # [16, 4, 128, 512] tiled
        out[core // ncb] += r.transpose(0, 2, 1, 3).reshape(S, D)
    return out



# revision 14
# speedup vs baseline: 1.1349x; 1.1349x over previous
"""Trainium2 Bass kernel for nn_MultiHeadAttention_41884521070801.

Sharding: tensor-parallel over heads (4 heads/core) x data-parallel over
batch (B=2) => 8 cores. Each core computes, for its batch element and its
4 heads: QKV projections (+RoPE), causal softmax attention (flash-style,
transposed-scores layout so no transposes are needed on-device), and its
partial output projection (rows of Wo^T). Host sums the 4 partial outputs
per batch element.

All matmuls run in bf16 with fp32 PSUM accumulation. RoPE and softmax
statistics are computed in fp32.

v2 schedule notes:
- attention uses one global (head, kblock) fifo with LOOKAHEAD so the PE
  never waits on the ACT exp, including across head boundaries
- the causal diagonal mask is a DVE multiply on the post-exp p-block
  (PE previously paid an identity-matmul accumulate per diagonal block)
- softmax normalization is recip (DVE) -> partition_broadcast (GPSIMD)
  -> multiply (DVE): no PE involvement
- Wo of chunk qc is emitted after proj of chunk qc+1 so the PE has a
  full projection phase of work while DVE finishes the last head's
  normalize
- x chunk loads are one DMA on the SP queue; output stores are one
  bf16 DMA per 128-row block on the ACT queue
"""

import math

import numpy as np
import ml_dtypes

import concourse.bacc as bacc
import concourse.tile as tile
from concourse import mybir
from concourse.bass_utils import run_bass_kernel_spmd

N_CORES = 8
B = 2
S = 2048
D = 2048
H = 16
HD = 128          # head dim
HLOC = 4          # heads per core
DLOC = HLOC * HD  # 512, per-core slice of the concat-head dim
QCH = 512         # q chunk size
NQC = S // QCH    # 4
NKB = S // 128    # 16 k-blocks
NEB = D // 128    # 16 e-blocks (contraction blocks for projections)
ROPE_THETA = 10000.0
NEG = -1.0e30

F32 = mybir.dt.float32
BF16 = mybir.dt.bfloat16

_BUILD_CACHE = {}

FLAGS = {
    "timing_io": False,   # all data in internal DRAM, tiny external I/O
    "norm_gpsimd": True,  # normalize via gpsimd partition_broadcast
    "mask_dve": True,     # causal diag mask via DVE mul (else PE matmul add)
    "lookahead": 4,       # scores/exp blocks in flight ahead of PV
}


def _emit_consts(nc, tc, pools, tensors):
    """Emit the one-time constant/weight loads.

    DMA queue order matters for the cold start: xc(0) is issued by the
    body right after these, so front-load only what the first Q/K chains
    and rope need (wq, small consts, cos/sin, wk), then wv/wo.
    """
    (consts, resid, xc_pool, ps_pool, work, p_pool, rb_pool, oc_pool,
     qcur_pool, ocur_pool) = pools
    (xT, wqT, wkT, wvT, woT, cosT, sinT, rT, amB, mask01, outp) = tensors

    consts.wq = consts.tile([128, NEB, DLOC], BF16, tag="wq", name="wq")
    consts.wk = consts.tile([128, NEB, DLOC], BF16, tag="wk", name="wk")
    consts.wv = consts.tile([128, NEB, DLOC], BF16, tag="wv", name="wv")
    consts.wo = consts.tile([128, HLOC, D], BF16, tag="wo", name="wo")
    # SP queue gets only wq + the tiny consts so the body's first xc load
    # starts right behind them; the rest rides the DVE queue in parallel.
    nc.sync.dma_start(out=consts.wq, in_=wqT[:].rearrange("e p d -> p e d"))
    consts.rT = consts.tile([128, HD], BF16, tag="rT", name="rTs")
    nc.sync.dma_start(out=consts.rT, in_=rT[:])
    consts.amB = consts.tile([128, NKB], F32, tag="amB", name="amBs")
    nc.sync.dma_start(out=consts.amB, in_=amB[:])
    consts.mask01 = consts.tile([128, 128], BF16, tag="mask01", name="mask01")
    nc.sync.dma_start(out=consts.mask01, in_=mask01[:])
    consts.cos = consts.tile([128, S], F32, tag="cos", name="cos")
    consts.sin = consts.tile([128, S], F32, tag="sin", name="sin")
    nc.scalar.dma_start(out=consts.wk, in_=wkT[:].rearrange("e p d -> p e d"))
    nc.scalar.dma_start(out=consts.cos, in_=cosT[:])
    nc.scalar.dma_start(out=consts.sin, in_=sinT[:])
    nc.scalar.dma_start(out=consts.wv, in_=wvT[:].rearrange("e p d -> p e d"))
    nc.scalar.dma_start(out=consts.wo, in_=woT[:].rearrange("h p d -> p h d"))
    consts.ones_bf = consts.tile([128, 1], BF16, tag="ones_bf", name="ones_bf")
    nc.vector.memset(consts.ones_bf, 1.0)
    consts.ones_row = consts.tile([1, 128], F32, tag="ones_row", name="ones_row")
    nc.vector.memset(consts.ones_row, 1.0)
    # persistent activations (K and V must stay for the whole pass)
    consts.kro = [resid.tile([128, S], BF16, tag=f"kro{h}", name=f"kro{h}")
                  for h in range(HLOC)]
    consts.v = [resid.tile([128, DLOC], BF16, tag=f"v{kb}", name=f"v{kb}")
                for kb in range(NKB)]


def _emit_body(nc, tc, pools, tensors):
    """Emit one full forward pass (consts already emitted)."""
    (consts, resid, xc_pool, ps_pool, work, p_pool, rb_pool, oc_pool,
     qcur_pool, ocur_pool) = pools
    (xT, wqT, wkT, wvT, woT, cosT, sinT, rT, amB, mask01, outp) = tensors

    wq, wk, wv, wo = consts.wq, consts.wk, consts.wv, consts.wo
    cos_s, sin_s, rT_s, amB_s = consts.cos, consts.sin, consts.rT, consts.amB
    kro, v_s = consts.kro, consts.v
    LOOKAHEAD = FLAGS["lookahead"]

    def rope_pre(src_ps):
        """ACT-copy psum -> bf16 sbuf (stage 1 of rope)."""
        qf = work.tile([128, QCH], BF16, tag="ropef", name="ropef", bufs=4)
        nc.scalar.copy(qf, src_ps)
        return qf

    def rope_rot(qf):
        """PE rotate-half matmul (stage 2)."""
        rot = ps_pool.tile([128, QCH], F32, tag="ps", name="ps")
        nc.tensor.matmul(rot, lhsT=rT_s, rhs=qf, start=True, stop=True)
        return rot

    def rope_fin(qf, rot, dst_ap, qc):
        """DVE combine (stage 3)."""
        t1 = work.tile([128, QCH], F32, tag="ropet1", name="ropet1", bufs=3)
        nc.vector.tensor_mul(t1, qf, cos_s[:, qc * QCH:(qc + 1) * QCH])
        t2 = work.tile([128, QCH], F32, tag="ropet2", name="ropet2", bufs=3)
        nc.vector.tensor_mul(t2, rot, sin_s[:, qc * QCH:(qc + 1) * QCH])
        nc.vector.tensor_add(dst_ap, t1, t2)

    norm_q = []   # (ops, sps, ot) awaiting normalize

    def emit_normalize():
        ops0, sps0, ot0 = norm_q.pop(0)
        r_row = rb_pool.tile([1, QCH], F32, tag="rrow", name="rrow")
        nc.vector.reciprocal(r_row, sps0)
        rb_sb = rb_pool.tile([128, QCH], F32, tag="rb", name="rb")
        if FLAGS["norm_gpsimd"]:
            nc.gpsimd.partition_broadcast(rb_sb, r_row, channels=128)
        else:
            rb_ps = ps_pool.tile([128, QCH], F32, tag="ps", name="ps")
            nc.tensor.matmul(rb_ps, lhsT=consts.ones_row, rhs=r_row,
                             start=True, stop=True)
            nc.vector.tensor_copy(rb_sb, rb_ps)
        nc.vector.tensor_mul(ot0[:], ops0, rb_sb)

    def emit_proj(qc, xc):
        """QKV projections + rope for chunk qc. Returns qcur (4 tiles)."""
        qcur = []
        chains = []
        for h in range(HLOC):
            qt = qcur_pool.tile([128, QCH], BF16, tag="qcur", name="qcur")
            qcur.append(qt)
            chains.append((wq, h, qt[:]))
        for h in range(HLOC):
            chains.append((wk, h, kro[h][:, qc * QCH:(qc + 1) * QCH]))

        pending = []  # (qf, dst_ap) awaiting rot+fin

        def drain_pending():
            qf, dst_ap = pending.pop(0)
            rot = rope_rot(qf)
            rope_fin(qf, rot, dst_ap, qc)

        for (w_s, h, dst_ap) in chains:
            pp = ps_pool.tile([128, QCH], F32, tag="ps", name="ps")
            for e in range(NEB):
                nc.tensor.matmul(
                    pp, lhsT=w_s[:, e, h * HD:(h + 1) * HD], rhs=xc[:, e, :],
                    start=(e == 0), stop=(e == NEB - 1))
            qf = rope_pre(pp)
            pending.append((qf, dst_ap))
            if len(pending) >= 2:
                drain_pending()

        for kb4 in range(4):
            kb = qc * 4 + kb4
            pp = ps_pool.tile([128, DLOC], F32, tag="ps", name="ps")
            for e in range(NEB):
                nc.tensor.matmul(
                    pp, lhsT=xc[:, e, kb4 * 128:(kb4 + 1) * 128],
                    rhs=wv[:, e, :],
                    start=(e == 0), stop=(e == NEB - 1))
            nc.vector.tensor_copy(v_s[kb], pp)
            while pending:
                drain_pending()
        while pending:
            drain_pending()
        return qcur

    def emit_attn(qc, qcur):
        """Causal attention for chunk qc. Returns ocur (4 tiles)."""
        nkb = 4 * qc + 4
        ocur = [ocur_pool.tile([128, QCH], BF16, tag="ocur", name="ocur")
                for _ in range(HLOC)]
        ops = {}
        # two PSUM banks hold the 4 heads' softmax denominators: head h in
        # tile h//2 at partition (h%2)*32 (matmul out base must be 0/32/64)
        sums_ps = [ps_pool.tile([33, QCH], F32, tag="ps", name="ps")
                   for _ in range(2)]

        def sums_row(h):
            r = (h % 2) * 32
            return sums_ps[h // 2][r:r + 1, :]

        def emit_scores(h, kb):
            off = max(0, (kb - 4 * qc) * 128)
            diag = kb >= 4 * qc
            s_ps = ps_pool.tile([128, QCH], F32, tag="ps", name="ps")
            nc.tensor.matmul(
                s_ps[:, off:], lhsT=kro[h][:, kb * 128:(kb + 1) * 128],
                rhs=qcur[h][:, off:], start=True, stop=True)
            p_sb = p_pool.tile([128, QCH], BF16, tag="p", name="p")
            nc.scalar.activation(
                p_sb[:, off:], s_ps[:, off:],
                mybir.ActivationFunctionType.Exp,
                bias=amB_s[:, kb:kb + 1], scale=1.0)
            if diag and FLAGS["mask_dve"]:
                nc.vector.tensor_mul(p_sb[:, off:off + 128],
                                     p_sb[:, off:off + 128], consts.mask01)
            return (h, kb, p_sb, off)

        seq = [(h, kb) for h in range(HLOC) for kb in range(nkb)]
        fifo = [emit_scores(h, kb) for (h, kb) in seq[:LOOKAHEAD]]
        for i, (h, kb) in enumerate(seq):
            if i + LOOKAHEAD < len(seq):
                fifo.append(emit_scores(*seq[i + LOOKAHEAD]))
            fh, fkb, p_sb, off = fifo.pop(0)
            assert (fh, fkb) == (h, kb)
            if kb == 0:
                ops[h] = ps_pool.tile([128, QCH], F32, tag="ps", name="ps")
            nc.tensor.matmul(
                ops[h][:, off:], lhsT=v_s[kb][:, h * HD:(h + 1) * HD],
                rhs=p_sb[:, off:],
                start=(kb == 0), stop=(kb == nkb - 1), skip_group_check=True)
            nc.tensor.matmul(
                sums_row(h)[:, off:], lhsT=consts.ones_bf, rhs=p_sb[:, off:],
                start=(kb == 0), stop=(kb == nkb - 1), skip_group_check=True)
            if kb == nkb - 1:
                norm_q.append((ops[h], sums_row(h), ocur[h]))
            if kb == 1 and norm_q:
                emit_normalize()
        # drain everything but keep the PE stream moving: these are
        # DVE/GPSIMD-only ops
        while norm_q:
            emit_normalize()
        return ocur

    def emit_wo(qc, ocur):
        """Output projection + store for chunk qc."""
        for qb4 in range(QCH // 128):
            qb = qc * 4 + qb4
            obuf = oc_pool.tile([128, D], BF16, tag="oc", name="oc")
            for ec in range(D // QCH):
                op_ps = ps_pool.tile([128, QCH], F32, tag="ps", name="ps")
                for h in range(HLOC):
                    nc.tensor.matmul(
                        op_ps,
                        lhsT=ocur[h][:, qb4 * 128:(qb4 + 1) * 128],
                        rhs=wo[:, h, ec * QCH:(ec + 1) * QCH],
                        start=(h == 0), stop=(h == HLOC - 1))
                dst = obuf[:, ec * QCH:(ec + 1) * QCH]
                if ec % 2 == 0:
                    nc.scalar.copy(dst, op_ps)
                else:
                    nc.vector.tensor_copy(dst, op_ps)
            nc.scalar.dma_start(out=outp[qb], in_=obuf)

    wo_pending = []
    for qc in range(NQC):
        xc = xc_pool.tile([128, NEB, QCH], BF16, tag="xc", name="xc")
        nc.sync.dma_start(out=xc, in_=xT[qc].rearrange("e p q -> p e q"))
        qcur = emit_proj(qc, xc)
        if wo_pending:
            emit_wo(*wo_pending.pop(0))
        ocur = emit_attn(qc, qcur)
        wo_pending.append((qc, ocur))
    emit_wo(*wo_pending.pop(0))


def build_nc(repeat=1, unroll=1):
    key = (repeat, unroll, tuple(sorted(FLAGS.items())))
    if key in _BUILD_CACHE:
        return _BUILD_CACHE[key]
    nc = bacc.Bacc("TRN2", target_bir_lowering=False, debug=False,
                   num_devices=N_CORES)
    if FLAGS["timing_io"]:
        kind = "Internal"
        dummy_in = nc.dram_tensor("dummy_in", [1, 4], F32, kind="ExternalInput")
        dummy_out = nc.dram_tensor("dummy_out", [1, 4], F32, kind="ExternalOutput")
    else:
        kind = "ExternalInput"
    xT = nc.dram_tensor("xT", [NQC, NEB, 128, QCH], BF16, kind=kind)
    wqT = nc.dram_tensor("wqT", [NEB, 128, DLOC], BF16, kind=kind)
    wkT = nc.dram_tensor("wkT", [NEB, 128, DLOC], BF16, kind=kind)
    wvT = nc.dram_tensor("wvT", [NEB, 128, DLOC], BF16, kind=kind)
    woT = nc.dram_tensor("woT", [HLOC, 128, D], BF16, kind=kind)
    cosT = nc.dram_tensor("cosT", [HD, S], F32, kind=kind)
    sinT = nc.dram_tensor("sinT", [HD, S], F32, kind=kind)
    rT = nc.dram_tensor("rT", [HD, HD], BF16, kind=kind)
    amB = nc.dram_tensor("amB", [128, NKB], F32, kind=kind)
    mask01 = nc.dram_tensor("mask01", [128, 128], BF16, kind=kind)
    if FLAGS["timing_io"]:
        outp = nc.dram_tensor("outp", [S // 128, 128, D], BF16, kind="Internal")
    else:
        outp = nc.dram_tensor("outp", [S // 128, 128, D], BF16,
                              kind="ExternalOutput")
    tensors = (xT, wqT, wkT, wvT, woT, cosT, sinT, rT, amB, mask01, outp)

    from contextlib import ExitStack
    with tile.TileContext(nc) as tc, ExitStack() as ctx:
        consts = ctx.enter_context(tc.tile_pool(name="consts", bufs=1))
        resid = ctx.enter_context(tc.tile_pool(name="resid", bufs=1))
        xc_pool = ctx.enter_context(tc.tile_pool(name="xc", bufs=2))
        ps_pool = ctx.enter_context(tc.tile_pool(name="ps", bufs=8, space="PSUM"))
        work = ctx.enter_context(tc.tile_pool(name="work", bufs=2))
        p_pool = ctx.enter_context(tc.tile_pool(name="p", bufs=6))
        rb_pool = ctx.enter_context(tc.tile_pool(name="rb", bufs=2))
        oc_pool = ctx.enter_context(tc.tile_pool(name="oc", bufs=3))
        qcur_pool = ctx.enter_context(tc.tile_pool(name="qcur", bufs=8))
        ocur_pool = ctx.enter_context(tc.tile_pool(name="ocur", bufs=8))
        pools = (consts, resid, xc_pool, ps_pool, work, p_pool, rb_pool,
                 oc_pool, qcur_pool, ocur_pool)
        _emit_consts(nc, tc, pools, tensors)
        if FLAGS["timing_io"]:
            dsb = pools[4].tile([1, 4], F32, tag="dummy", name="dummy")
            nc.sync.dma_start(out=dsb, in_=dummy_in[:])
            nc.sync.dma_start(out=dummy_out[:], in_=dsb)
        if repeat == 1:
            for _ in range(unroll):
                _emit_body(nc, tc, pools, tensors)
        else:
            with tc.For_i(0, repeat, 1, hint_engines=(
                    mybir.EngineType.PE, mybir.EngineType.DVE,
                    mybir.EngineType.Activation)):
                _emit_body(nc, tc, pools, tensors)
    nc.compile()
    _BUILD_CACHE[key] = nc
    return nc


def make_core_inputs(hidden_states, attention_mask, Wq, Wk, Wv, Wo):
    """Host-side prep: returns list of 8 in_maps."""
    f32 = np.float32
    bf16 = ml_dtypes.bfloat16
    hidden_states = np.asarray(hidden_states, dtype=f32)
    attention_mask = np.asarray(attention_mask, dtype=f32)
    Wq = np.asarray(Wq, dtype=f32)
    Wk = np.asarray(Wk, dtype=f32)
    Wv = np.asarray(Wv, dtype=f32)
    Wo = np.asarray(Wo, dtype=f32)

    # rope tables, [hd, S] layout
    invf = 1.0 / (ROPE_THETA ** (np.arange(0, HD, 2, dtype=f32) / HD))
    t = np.arange(S, dtype=f32)
    fr = t[:, None] * invf[None, :]            # [S, hd/2]
    emb = np.concatenate([fr, fr], axis=-1)    # [S, hd]
    cosT = np.cos(emb).T.astype(f32).copy()    # [hd, S]
    sinT = np.sin(emb).T.astype(f32).copy()

    # rotate-half matrix: (R @ x)[i] = -x[i+64] (i<64), x[i-64] (i>=64)
    R = np.zeros((HD, HD), dtype=f32)
    half = HD // 2
    for i in range(half):
        R[i, i + half] = -1.0
        R[i + half, i] = 1.0
    rT = R.T.copy()

    # 0/1 visibility mask for the diagonal 128x128 block: key row p visible
    # to query col c iff p <= c
    p = np.arange(128)[:, None]
    c = np.arange(128)[None, :]
    mask01 = (p <= c).astype(np.float32).astype(bf16)

    scale = 1.0 / math.sqrt(HD)
    in_maps = []
    for core in range(N_CORES):
        b = core // (N_CORES // B)
        hg = core % (N_CORES // B)
        rows = slice(hg * DLOC, (hg + 1) * DLOC)
        amv = np.where(attention_mask[b] == 0, NEG, attention_mask[b]).astype(f32)
        in_maps.append({
            "xT": np.ascontiguousarray(
                hidden_states[b].T.reshape(NEB, 128, NQC, QCH)
                .transpose(2, 0, 1, 3)).astype(bf16),
            "wqT": (Wq[rows, :] * scale).T.reshape(NEB, 128, DLOC).astype(bf16),
            "wkT": Wk[rows, :].T.reshape(NEB, 128, DLOC).astype(bf16),
            "wvT": Wv[rows, :].T.reshape(NEB, 128, DLOC).astype(bf16),
            "woT": Wo[:, rows].T.reshape(HLOC, 128, D).astype(bf16),
            "cosT": cosT,
            "sinT": sinT,
            "rT": rT.astype(bf16),
            "amB": amv.reshape(NKB, 128).T.copy(),
            "mask01": mask01,
        })
    return in_maps


def kernel(**inputs):
    nc = build_nc()
    in_maps = make_core_inputs(**inputs)
    res = run_bass_kernel_spmd(nc, in_maps, list(range(N_CORES)))
    out = np.zeros((B, S, D), dtype=np.float32)
    ncb = N_CORES // B
    for core in range(N_CORES):
        r = res.results[core]["outp"]          # [16, 128, 2048] bf16
        out[core // ncb] += np.asarray(r, dtype=np.float32).reshape(S, D)
    return out


# revision 24
# speedup vs baseline: 1.2361x; 1.0892x over previous
"""Trainium2 Bass kernel for nn_MultiHeadAttention_41884521070801.

Sharding: tensor-parallel over heads (4 heads/core) x data-parallel over
batch (B=2) => 8 cores. Each core computes, for its batch element and its
4 heads: QKV projections (+RoPE), causal softmax attention (flash-style,
transposed-scores layout so no transposes are needed on-device), and its
partial output projection (rows of Wo^T). Host sums the 4 partial outputs
per batch element.

All matmuls run in bf16 with fp32 PSUM accumulation. RoPE and softmax
statistics are computed in fp32.

v2 schedule notes:
- attention uses one global (head, kblock) fifo with LOOKAHEAD so the PE
  never waits on the ACT exp, including across head boundaries
- the causal diagonal mask is a DVE multiply on the post-exp p-block
  (PE previously paid an identity-matmul accumulate per diagonal block)
- softmax normalization is recip (DVE) -> partition_broadcast (GPSIMD)
  -> multiply (DVE): no PE involvement
- Wo of chunk qc is emitted after proj of chunk qc+1 so the PE has a
  full projection phase of work while DVE finishes the last head's
  normalize
- x chunk loads are one DMA on the SP queue; output stores are one
  bf16 DMA per 128-row block on the ACT queue
"""

import math

import numpy as np
import ml_dtypes

import concourse.bacc as bacc
import concourse.tile as tile
from concourse import mybir
from concourse.bass_utils import run_bass_kernel_spmd

N_CORES = 8
B = 2
S = 2048
D = 2048
H = 16
HD = 128          # head dim
HLOC = 4          # heads per core
DLOC = HLOC * HD  # 512, per-core slice of the concat-head dim
QCH = 512         # q chunk size
NQC = S // QCH    # 4
NKB = S // 128    # 16 k-blocks
NEB = D // 128    # 16 e-blocks (contraction blocks for projections)
ROPE_THETA = 10000.0
NEG = -1.0e30

F32 = mybir.dt.float32
BF16 = mybir.dt.bfloat16

_BUILD_CACHE = {}

FLAGS = {
    "timing_io": False,   # all data in internal DRAM, tiny external I/O
    "norm_gpsimd": True,  # normalize via gpsimd partition_broadcast
    "mask_dve": True,     # causal diag mask via DVE mul (else PE matmul add)
    "lookahead": 4,       # scores/exp blocks in flight ahead of PV
}


def _emit_consts(nc, tc, pools, tensors):
    """Emit the one-time constant/weight loads.

    DMA queue order matters for the cold start: xc(0) is issued by the
    body right after these, so front-load only what the first Q/K chains
    and rope need (wq, small consts, cos/sin, wk), then wv/wo.
    """
    (consts, resid, xc_pool, ps_pool, work, p_pool, rb_pool, oc_pool,
     qcur_pool, ocur_pool) = pools
    (xT, wqT, wkT, wvT, woT, cosT, sinT, rT, amB, mask01, outp) = tensors

    consts.wq = consts.tile([128, NEB, DLOC], BF16, tag="wq", name="wq")
    consts.wk = consts.tile([128, NEB, DLOC], BF16, tag="wk", name="wk")
    consts.wv = consts.tile([128, NEB, DLOC], BF16, tag="wv", name="wv")
    consts.wo = consts.tile([128, HLOC, D], BF16, tag="wo", name="wo")
    # SP queue gets only wq + the tiny consts so the body's first xc load
    # starts right behind them; the rest rides the ACT queue in parallel.
    # wq in quarters so the first chain starts after 1/4 of it arrives.
    for g in range(4):
        nc.sync.dma_start(out=consts.wq[:, g * 4:(g + 1) * 4, :],
                          in_=wqT[g * 4:(g + 1) * 4].rearrange("e p d -> p e d"))
    consts.rT = consts.tile([128, HD], BF16, tag="rT", name="rTs")
    nc.sync.dma_start(out=consts.rT, in_=rT[:])
    consts.amB = consts.tile([128, NKB], F32, tag="amB", name="amBs")
    nc.sync.dma_start(out=consts.amB, in_=amB[:])
    consts.mask01 = consts.tile([128, 128], BF16, tag="mask01", name="mask01")
    nc.sync.dma_start(out=consts.mask01, in_=mask01[:])
    consts.cos = consts.tile([128, S], F32, tag="cos", name="cos")
    consts.sin = consts.tile([128, S], F32, tag="sin", name="sin")
    for g in range(2):
        nc.scalar.dma_start(out=consts.wk[:, g * 8:(g + 1) * 8, :],
                            in_=wkT[g * 8:(g + 1) * 8].rearrange("e p d -> p e d"))
    nc.scalar.dma_start(out=consts.cos, in_=cosT[:])
    nc.scalar.dma_start(out=consts.sin, in_=sinT[:])
    nc.scalar.dma_start(out=consts.wv, in_=wvT[:].rearrange("e p d -> p e d"))
    nc.scalar.dma_start(out=consts.wo, in_=woT[:].rearrange("h p d -> p h d"))
    consts.ones_bf = consts.tile([128, 1], BF16, tag="ones_bf", name="ones_bf")
    nc.vector.memset(consts.ones_bf, 1.0)
    consts.ones_row = consts.tile([1, 128], F32, tag="ones_row", name="ones_row")
    nc.vector.memset(consts.ones_row, 1.0)
    # persistent activations (K and V must stay for the whole pass)
    consts.kro = [resid.tile([128, S], BF16, tag=f"kro{h}", name=f"kro{h}")
                  for h in range(HLOC)]
    consts.v = [resid.tile([128, DLOC], BF16, tag=f"v{kb}", name=f"v{kb}")
                for kb in range(NKB)]


def _emit_body(nc, tc, pools, tensors):
    """Emit one full forward pass (consts already emitted)."""
    (consts, resid, xc_pool, ps_pool, work, p_pool, rb_pool, oc_pool,
     qcur_pool, ocur_pool) = pools
    (xT, wqT, wkT, wvT, woT, cosT, sinT, rT, amB, mask01, outp) = tensors

    wq, wk, wv, wo = consts.wq, consts.wk, consts.wv, consts.wo
    cos_s, sin_s, amB_s = consts.cos, consts.sin, consts.amB
    kro, v_s = consts.kro, consts.v
    LOOKAHEAD = FLAGS["lookahead"]

    def rope_pre(src_ps):
        """ACT-copy psum -> bf16 sbuf (stage 1 of rope)."""
        qf = work.tile([128, QCH], BF16, tag="ropef", name="ropef", bufs=4)
        nc.scalar.copy(qf, src_ps)
        return qf

    def rope_rot(qf):
        """PE rotate-half matmul (only PE can move data across partitions)."""
        rot = ps_pool.tile([128, QCH], F32, tag="ps", name="ps")
        nc.tensor.matmul(rot, lhsT=consts.rT, rhs=qf, start=True, stop=True)
        return rot

    def rope_fin(qf, dst_ap, qc):
        """DVE combine: dst = qf*cos + rot(qf)*sin."""
        rot = rope_rot(qf)
        t1 = work.tile([128, QCH], F32, tag="ropet1", name="ropet1", bufs=3)
        nc.vector.tensor_mul(t1, qf, cos_s[:, qc * QCH:(qc + 1) * QCH])
        t2 = work.tile([128, QCH], F32, tag="ropet2", name="ropet2", bufs=3)
        nc.vector.tensor_mul(t2, rot, sin_s[:, qc * QCH:(qc + 1) * QCH])
        nc.vector.tensor_add(dst_ap, t1, t2)

    norm_q = []   # (ops, sps, ot) awaiting normalize

    def emit_normalize():
        ops0, sps0, ot0 = norm_q.pop(0)
        r_row = rb_pool.tile([1, QCH], F32, tag="rrow", name="rrow")
        nc.vector.reciprocal(r_row, sps0)
        rb_sb = rb_pool.tile([128, QCH], F32, tag="rb", name="rb")
        if FLAGS["norm_gpsimd"]:
            nc.gpsimd.partition_broadcast(rb_sb, r_row, channels=128)
        else:
            rb_ps = ps_pool.tile([128, QCH], F32, tag="ps", name="ps")
            nc.tensor.matmul(rb_ps, lhsT=consts.ones_row, rhs=r_row,
                             start=True, stop=True)
            nc.vector.tensor_copy(rb_sb, rb_ps)
        nc.vector.tensor_mul(ot0[:], ops0, rb_sb)

    def emit_proj(qc, xc):
        """QKV projections + rope for chunk qc. Returns qcur (4 tiles)."""
        qcur = []
        chains = []
        for h in range(HLOC):
            qt = qcur_pool.tile([128, QCH], BF16, tag="qcur", name="qcur")
            qcur.append(qt)
            chains.append((wq, h, qt[:]))
        for h in range(HLOC):
            chains.append((wk, h, kro[h][:, qc * QCH:(qc + 1) * QCH]))

        pending = []  # (qf, dst_ap) awaiting fin

        def drain_pending():
            qf, dst_ap = pending.pop(0)
            rope_fin(qf, dst_ap, qc)

        for (w_s, h, dst_ap) in chains:
            pp = ps_pool.tile([128, QCH], F32, tag="ps", name="ps")
            for e in range(NEB):
                nc.tensor.matmul(
                    pp, lhsT=w_s[:, e, h * HD:(h + 1) * HD], rhs=xc[:, e, :],
                    start=(e == 0), stop=(e == NEB - 1))
            qf = rope_pre(pp)
            pending.append((qf, dst_ap))
            if len(pending) >= 2:
                drain_pending()

        for kb4 in range(4):
            kb = qc * 4 + kb4
            pp = ps_pool.tile([128, DLOC], F32, tag="ps", name="ps")
            for e in range(NEB):
                nc.tensor.matmul(
                    pp, lhsT=xc[:, e, kb4 * 128:(kb4 + 1) * 128],
                    rhs=wv[:, e, :],
                    start=(e == 0), stop=(e == NEB - 1))
            nc.vector.tensor_copy(v_s[kb], pp)
            while pending:
                drain_pending()
        while pending:
            drain_pending()
        return qcur

    def emit_attn(qc, qcur, mid=None):
        """Causal attention for chunk qc. Returns ocur (4 tiles).

        ``mid`` (if given) is emitted right after the scores fifo warmup,
        giving the ACT engine a head start on the first exps while the PE
        chews through ``mid``'s matmuls (used for the previous chunk's Wo).
        """
        nkb = 4 * qc + 4
        ocur = [ocur_pool.tile([128, QCH], BF16, tag="ocur", name="ocur")
                for _ in range(HLOC)]
        ops = {}
        # two PSUM banks hold the 4 heads' softmax denominators: head h in
        # tile h//2 at partition (h%2)*32 (matmul out base must be 0/32/64)
        sums_ps = [ps_pool.tile([33, QCH], F32, tag="ps", name="ps")
                   for _ in range(2)]

        def sums_row(h):
            r = (h % 2) * 32
            return sums_ps[h // 2][r:r + 1, :]

        def emit_scores(h, kb):
            off = max(0, (kb - 4 * qc) * 128)
            diag = kb >= 4 * qc
            s_ps = ps_pool.tile([128, QCH], F32, tag="ps", name="ps")
            nc.tensor.matmul(
                s_ps[:, off:], lhsT=kro[h][:, kb * 128:(kb + 1) * 128],
                rhs=qcur[h][:, off:], start=True, stop=True)
            p_sb = p_pool.tile([128, QCH], BF16, tag="p", name="p")
            nc.scalar.activation(
                p_sb[:, off:], s_ps[:, off:],
                mybir.ActivationFunctionType.Exp,
                bias=amB_s[:, kb:kb + 1], scale=1.0)
            if diag and FLAGS["mask_dve"]:
                nc.vector.tensor_mul(p_sb[:, off:off + 128],
                                     p_sb[:, off:off + 128], consts.mask01)
            return (h, kb, p_sb, off)

        seq = [(h, kb) for h in range(HLOC) for kb in range(nkb)]
        fifo = [emit_scores(h, kb) for (h, kb) in seq[:LOOKAHEAD]]
        if mid is not None:
            mid()
        for i, (h, kb) in enumerate(seq):
            if i + LOOKAHEAD < len(seq):
                fifo.append(emit_scores(*seq[i + LOOKAHEAD]))
            fh, fkb, p_sb, off = fifo.pop(0)
            assert (fh, fkb) == (h, kb)
            if kb == 0:
                ops[h] = ps_pool.tile([128, QCH], F32, tag="ps", name="ps")
            nc.tensor.matmul(
                ops[h][:, off:], lhsT=v_s[kb][:, h * HD:(h + 1) * HD],
                rhs=p_sb[:, off:],
                start=(kb == 0), stop=(kb == nkb - 1), skip_group_check=True)
            nc.tensor.matmul(
                sums_row(h)[:, off:], lhsT=consts.ones_bf, rhs=p_sb[:, off:],
                start=(kb == 0), stop=(kb == nkb - 1), skip_group_check=True)
            if kb == nkb - 1:
                norm_q.append((ops[h], sums_row(h), ocur[h]))
            if kb == 1 and norm_q:
                emit_normalize()
        # drain everything but keep the PE stream moving: these are
        # DVE/GPSIMD-only ops
        while norm_q:
            emit_normalize()
        return ocur

    def emit_wo(qc, ocur):
        """Output projection + store for chunk qc."""
        for qb4 in range(QCH // 128):
            qb = qc * 4 + qb4
            obuf = oc_pool.tile([128, D], BF16, tag="oc", name="oc")
            for ec in range(D // QCH):
                op_ps = ps_pool.tile([128, QCH], F32, tag="ps", name="ps")
                for h in range(HLOC):
                    nc.tensor.matmul(
                        op_ps,
                        lhsT=ocur[h][:, qb4 * 128:(qb4 + 1) * 128],
                        rhs=wo[:, h, ec * QCH:(ec + 1) * QCH],
                        start=(h == 0), stop=(h == HLOC - 1))
                dst = obuf[:, ec * QCH:(ec + 1) * QCH]
                if ec % 2 == 0:
                    nc.scalar.copy(dst, op_ps)
                else:
                    nc.vector.tensor_copy(dst, op_ps)
            nc.scalar.dma_start(out=outp[qb], in_=obuf)

    wo_pending = []
    for qc in range(NQC):
        xc = xc_pool.tile([128, NEB, QCH], BF16, tag="xc", name="xc")
        # quarters so the cold-start projection can begin after 1/4 arrives
        for g in range(4):
            nc.sync.dma_start(
                out=xc[:, g * 4:(g + 1) * 4, :],
                in_=xT[qc, g * 4:(g + 1) * 4].rearrange("e p q -> p e q"))
        qcur = emit_proj(qc, xc)
        mid = None
        if wo_pending:
            prev = wo_pending.pop(0)
            mid = lambda prev=prev: emit_wo(*prev)
        ocur = emit_attn(qc, qcur, mid=mid)
        wo_pending.append((qc, ocur))
    emit_wo(*wo_pending.pop(0))


def build_nc(repeat=1, unroll=1):
    key = (repeat, unroll, tuple(sorted(FLAGS.items())))
    if key in _BUILD_CACHE:
        return _BUILD_CACHE[key]
    nc = bacc.Bacc("TRN2", target_bir_lowering=False, debug=False,
                   num_devices=N_CORES)
    if FLAGS["timing_io"]:
        kind = "Internal"
        dummy_in = nc.dram_tensor("dummy_in", [1, 4], F32, kind="ExternalInput")
        dummy_out = nc.dram_tensor("dummy_out", [1, 4], F32, kind="ExternalOutput")
    else:
        kind = "ExternalInput"
    xT = nc.dram_tensor("xT", [NQC, NEB, 128, QCH], BF16, kind=kind)
    wqT = nc.dram_tensor("wqT", [NEB, 128, DLOC], BF16, kind=kind)
    wkT = nc.dram_tensor("wkT", [NEB, 128, DLOC], BF16, kind=kind)
    wvT = nc.dram_tensor("wvT", [NEB, 128, DLOC], BF16, kind=kind)
    woT = nc.dram_tensor("woT", [HLOC, 128, D], BF16, kind=kind)
    cosT = nc.dram_tensor("cosT", [HD, S], F32, kind=kind)
    sinT = nc.dram_tensor("sinT", [HD, S], F32, kind=kind)
    rT = nc.dram_tensor("rT", [HD, HD], BF16, kind=kind)
    amB = nc.dram_tensor("amB", [128, NKB], F32, kind=kind)
    mask01 = nc.dram_tensor("mask01", [128, 128], BF16, kind=kind)
    if FLAGS["timing_io"]:
        outp = nc.dram_tensor("outp", [S // 128, 128, D], BF16, kind="Internal")
    else:
        outp = nc.dram_tensor("outp", [S // 128, 128, D], BF16,
                              kind="ExternalOutput")
    tensors = (xT, wqT, wkT, wvT, woT, cosT, sinT, rT, amB, mask01, outp)

    from contextlib import ExitStack
    with tile.TileContext(nc) as tc, ExitStack() as ctx:
        consts = ctx.enter_context(tc.tile_pool(name="consts", bufs=1))
        resid = ctx.enter_context(tc.tile_pool(name="resid", bufs=1))
        xc_pool = ctx.enter_context(tc.tile_pool(name="xc", bufs=2))
        ps_pool = ctx.enter_context(tc.tile_pool(name="ps", bufs=8, space="PSUM"))
        work = ctx.enter_context(tc.tile_pool(name="work", bufs=2))
        p_pool = ctx.enter_context(tc.tile_pool(name="p", bufs=6))
        rb_pool = ctx.enter_context(tc.tile_pool(name="rb", bufs=2))
        oc_pool = ctx.enter_context(tc.tile_pool(name="oc", bufs=3))
        qcur_pool = ctx.enter_context(tc.tile_pool(name="qcur", bufs=8))
        ocur_pool = ctx.enter_context(tc.tile_pool(name="ocur", bufs=8))
        pools = (consts, resid, xc_pool, ps_pool, work, p_pool, rb_pool,
                 oc_pool, qcur_pool, ocur_pool)
        _emit_consts(nc, tc, pools, tensors)
        if FLAGS["timing_io"]:
            dsb = pools[4].tile([1, 4], F32, tag="dummy", name="dummy")
            nc.sync.dma_start(out=dsb, in_=dummy_in[:])
            nc.sync.dma_start(out=dummy_out[:], in_=dsb)
        if repeat == 1:
            for _ in range(unroll):
                _emit_body(nc, tc, pools, tensors)
        else:
            assert repeat % unroll == 0
            with tc.For_i(0, repeat // unroll, 1, hint_engines=(
                    mybir.EngineType.PE, mybir.EngineType.DVE,
                    mybir.EngineType.Activation)):
                for _ in range(unroll):
                    _emit_body(nc, tc, pools, tensors)
    nc.compile()
    _BUILD_CACHE[key] = nc
    return nc


def make_core_inputs(hidden_states, attention_mask, Wq, Wk, Wv, Wo):
    """Host-side prep: returns list of 8 in_maps."""
    f32 = np.float32
    bf16 = ml_dtypes.bfloat16
    hidden_states = np.asarray(hidden_states, dtype=f32)
    attention_mask = np.asarray(attention_mask, dtype=f32)
    Wq = np.asarray(Wq, dtype=f32)
    Wk = np.asarray(Wk, dtype=f32)
    Wv = np.asarray(Wv, dtype=f32)
    Wo = np.asarray(Wo, dtype=f32)

    # rope tables, [hd, S] layout
    invf = 1.0 / (ROPE_THETA ** (np.arange(0, HD, 2, dtype=f32) / HD))
    t = np.arange(S, dtype=f32)
    fr = t[:, None] * invf[None, :]            # [S, hd/2]
    emb = np.concatenate([fr, fr], axis=-1)    # [S, hd]
    cosT = np.cos(emb).T.astype(f32).copy()    # [hd, S]
    sinT = np.sin(emb).T.astype(f32).copy()

    # rotate-half matrix: (R @ x)[i] = -x[i+64] (i<64), x[i-64] (i>=64)
    R = np.zeros((HD, HD), dtype=f32)
    half = HD // 2
    for i in range(half):
        R[i, i + half] = -1.0
        R[i + half, i] = 1.0
    rT = R.T.copy()

    # 0/1 visibility mask for the diagonal 128x128 block: key row p visible
    # to query col c iff p <= c
    p = np.arange(128)[:, None]
    c = np.arange(128)[None, :]
    mask01 = (p <= c).astype(np.float32).astype(bf16)

    scale = 1.0 / math.sqrt(HD)
    in_maps = []
    for core in range(N_CORES):
        b = core // (N_CORES // B)
        hg = core % (N_CORES // B)
        rows = slice(hg * DLOC, (hg + 1) * DLOC)
        amv = np.where(attention_mask[b] == 0, NEG, attention_mask[b]).astype(f32)
        in_maps.append({
            "xT": np.ascontiguousarray(
                hidden_states[b].T.reshape(NEB, 128, NQC, QCH)
                .transpose(2, 0, 1, 3)).astype(bf16),
            "wqT": (Wq[rows, :] * scale).T.reshape(NEB, 128, DLOC).astype(bf16),
            "wkT": Wk[rows, :].T.reshape(NEB, 128, DLOC).astype(bf16),
            "wvT": Wv[rows, :].T.reshape(NEB, 128, DLOC).astype(bf16),
            "woT": Wo[:, rows].T.reshape(HLOC, 128, D).astype(bf16),
            "rT": rT.astype(bf16),
            "cosT": cosT,
            "sinT": sinT,
            "amB": amv.reshape(NKB, 128).T.copy(),
            "mask01": mask01,
        })
    return in_maps


def kernel(**inputs):
    nc = build_nc()
    in_maps = make_core_inputs(**inputs)
    res = run_bass_kernel_spmd(nc, in_maps, list(range(N_CORES)))
    out = np.zeros((B, S, D), dtype=np.float32)
    ncb = N_CORES // B
    for core in range(N_CORES):
        r = res.results[core]["outp"]          # [16, 128, 2048] bf16
        out[core // ncb] += np.asarray(r, dtype=np.float32).reshape(S, D)
    return out
